# revision 5
# baseline (speedup 1.0000x reference)
"""MoE feed-forward (8 experts, top-2, SwiGLU) on 8 Trainium2 NeuronCores.

Strategy: expert parallelism with host routing. Core c owns expert c and
computes its expert's SwiGLU for the tokens routed to it (host-gathered,
feature-major). All matmul operands are bf16 (rel err ~3e-3, well under the
2e-2 gate) which halves DMA traffic vs fp32r and enables the PE's fast
weight load path. Gating probabilities are folded host-side into a second,
pre-scaled copy of x that feeds the up-projection, so the device applies
gating for free: y_c = Wd @ (silu(Wg x) * (Wu (g*x))). Output is fp16;
host scatter-adds the two expert contributions per token.
"""

import os
import sys
import time

sys.path.insert(0, "/opt/trn_rl_repo")

import numpy as np
import ml_dtypes

# ---------------------------------------------------------------------------
# Problem constants (hardcoded per contract)
B, S, D, E, I, TOPK = 2, 2048, 1024, 8, 1408, 2
T = B * S  # 4096 tokens
P = 128
D_T = D // P   # 8 d-tiles
I_T = I // P   # 11 i-tiles
N_CORES = 8
PSUM_MAX = 512  # fp32 elems per partition per PSUM bank

_VERBOSE = bool(int(os.environ.get("KERNEL_VERBOSE", "0")))


def _log(msg):
    if _VERBOSE:
        print(f"[kernel] {msg}", flush=True)


def _install_profile_shim():
    """Best-effort: make antenv.axon_hooks importable so trace=True works."""
    try:
        import antenv.axon_hooks  # noqa: F401
        return  # real module (or prior shim) present
    except Exception:
        pass
    try:
        sys.path.insert(0, os.path.dirname(os.path.abspath(__file__)))
        import axon_profile_shim
        axon_profile_shim.install()
    except Exception as exc:  # profiling is best-effort
        _log(f"profile shim unavailable: {exc}")


def host_gating(x2d: np.ndarray, gate_w: np.ndarray):
    """Exact router: scores -> top-2 -> softmax. Returns gating [T, E] fp32."""
    scores = x2d.astype(np.float64) @ gate_w.astype(np.float64).T  # [T, E]
    idx = np.argsort(-scores, axis=-1, kind="stable")[:, :TOPK]  # [T, 2]
    top = np.take_along_axis(scores, idx, axis=-1)  # [T, 2] descending
    m = top[:, :1]
    ex = np.exp(top - m)
    probs = ex / ex.sum(axis=-1, keepdims=True)  # [T, 2]
    gating = np.zeros((x2d.shape[0], E), dtype=np.float64)
    np.put_along_axis(gating, idx, probs, axis=-1)
    return gating.astype(np.float32)


def bf16(a: np.ndarray) -> np.ndarray:
    return np.ascontiguousarray(a, dtype=np.float32).astype(ml_dtypes.bfloat16)


def chunk_widths(max_n: int) -> list:
    """Split max_n tokens into PSUM-bank-sized chunks (multiples of 16).
    First chunk maximal (hides the weight-stream window), later chunks
    balanced so the last one (the output-DMA tail) stays small."""
    units = -(-max_n // 16)
    widths = []
    while units * 16 > PSUM_MAX:
        n_left = -(-units * 16 // PSUM_MAX)
        if n_left <= 2:
            break
        widths.append(PSUM_MAX)
        units -= PSUM_MAX // 16
    if units * 16 > PSUM_MAX:
        hi = -(-units // 2)
        widths += [hi * 16, (units - hi) * 16]
    elif units:
        widths.append(units * 16)
    return widths


# ---------------------------------------------------------------------------
# Bass kernel builder


def build_nc_routed(widths, n_cores=N_CORES):
    """Each core computes its expert for `sum(widths)` routed tokens
    (host-gathered, feature-major), in chunks of widths[i] tokens."""
    import concourse.mybir as mybir
    import concourse.tile as tile
    from concourse import bacc

    f32 = mybir.dt.float32
    bf = mybir.dt.bfloat16
    f16 = mybir.dt.float16
    cap = sum(widths)
    w0 = widths[0]

    nc = bacc.Bacc("TRN2", debug=False, num_devices=n_cores)

    xcT_d = nc.dram_tensor("xcT", [D, cap], bf, kind="ExternalInput")
    xgT_d = nc.dram_tensor("xgT", [D, cap], bf, kind="ExternalInput")
    wgT_d = nc.dram_tensor("wgT", [D, I], bf, kind="ExternalInput")
    wuT_d = nc.dram_tensor("wuT", [D, I], bf, kind="ExternalInput")
    wdT_d = nc.dram_tensor("wdT", [I, D], bf, kind="ExternalInput")
    ycomp_d = nc.dram_tensor("ycomp", [D, cap], f16, kind="ExternalOutput")

    xcT_r = xcT_d.ap().rearrange("(do dp) t -> dp do t", dp=P)
    xgT_r = xgT_d.ap().rearrange("(do dp) t -> dp do t", dp=P)
    wgT_r = wgT_d.ap().rearrange("(do dp) i -> dp do i", dp=P)
    wuT_r = wuT_d.ap().rearrange("(do dp) i -> dp do i", dp=P)
    wdT_r = wdT_d.ap().rearrange("(io ip) d -> ip io d", ip=P)
    ycomp_r = ycomp_d.ap().rearrange("(do dp) t -> dp do t", dp=P)

    starts = [sum(widths[:i]) for i in range(len(widths))]

    with tile.TileContext(nc) as tcx:
        with tcx.tile_pool(name="wpool", bufs=1) as wpool, \
             tcx.tile_pool(name="hpool", bufs=2) as hpool, \
             tcx.tile_pool(name="ypool", bufs=2) as ypool, \
             tcx.tile_pool(name="gspool", bufs=I_T + 1) as gspool, \
             tcx.tile_pool(name="psg", bufs=2, space="PSUM") as psg, \
             tcx.tile_pool(name="psu", bufs=2, space="PSUM") as psu, \
             tcx.tile_pool(name="psy", bufs=2, space="PSUM") as psy:

            wg_sb = wpool.tile([P, D_T, I], bf)
            wu_sb = wpool.tile([P, D_T, I], bf)
            wd_sb = wpool.tile([P, I_T, D], bf)
            xt = wpool.tile([P, D_T, cap], bf)
            xg = wpool.tile([P, D_T, cap], bf)

            # DMA issue order tuned for time-to-first-matmul and the PE's
            # consumption order (all-gate, then all-up, then down, per
            # chunk). Full-row transfers maximize per-descriptor bytes; the
            # x tensors load their full [d_o, :] row (all chunks at once).
            for d_o in range(D_T):
                nc.sync.dma_start(xt[:, d_o, :], xcT_r[:, d_o, :])
                nc.sync.dma_start(wg_sb[:, d_o, :], wgT_r[:, d_o, :])
            for d_o in range(D_T):
                nc.sync.dma_start(xg[:, d_o, :], xgT_r[:, d_o, :])
                nc.sync.dma_start(wu_sb[:, d_o, :], wuT_r[:, d_o, :])
            for i_o in range(I_T):
                nc.sync.dma_start(wd_sb[:, i_o, :], wdT_r[:, i_o, :])

            for ci, w in enumerate(widths):
                t0 = starts[ci]

                h = hpool.tile([P, I_T, w0], bf, tag="h")
                gs_tiles = []
                for i_o in range(I_T):
                    pg = psg.tile([P, w0], f32, tag="pg")
                    for d_o in range(D_T):
                        nc.tensor.matmul(
                            pg[:, :w], wg_sb[:, d_o, i_o * P:(i_o + 1) * P],
                            xt[:, d_o, t0:t0 + w],
                            start=(d_o == 0), stop=(d_o == D_T - 1))
                    gs = gspool.tile([P, w0], bf, tag="gs")
                    nc.scalar.activation(gs[:, :w], pg[:, :w],
                                         mybir.ActivationFunctionType.Silu)
                    gs_tiles.append(gs)
                for i_o in range(I_T):
                    pu = psu.tile([P, w0], f32, tag="pu")
                    for d_o in range(D_T):
                        nc.tensor.matmul(
                            pu[:, :w], wu_sb[:, d_o, i_o * P:(i_o + 1) * P],
                            xg[:, d_o, t0:t0 + w],
                            start=(d_o == 0), stop=(d_o == D_T - 1))
                    nc.vector.tensor_mul(out=h[:, i_o, :w],
                                         in0=gs_tiles[i_o][:, :w],
                                         in1=pu[:, :w])

                yout = ypool.tile([P, D_T, w0], f16, tag="yout")
                for d_o in range(D_T):
                    py = psy.tile([P, w0], f32, tag="py")
                    for i_o in range(I_T):
                        nc.tensor.matmul(
                            py[:, :w], wd_sb[:, i_o, d_o * P:(d_o + 1) * P],
                            h[:, i_o, :w],
                            start=(i_o == 0), stop=(i_o == I_T - 1))
                    nc.scalar.copy(out=yout[:, d_o, :w], in_=py[:, :w])
                    nc.scalar.dma_start(ycomp_r[:, d_o, t0:t0 + w],
                                        yout[:, d_o, :w])

    nc.compile()
    return nc


# ---------------------------------------------------------------------------
# Host-side wrapper

_CACHED = {}


def _get_nc_routed(widths):
    key = ("routed", tuple(widths))
    if key not in _CACHED:
        t0 = time.time()
        _CACHED[key] = build_nc_routed(list(widths))
        _log(f"built routed bass program (widths={widths}) "
             f"in {time.time() - t0:.1f}s")
    return _CACHED[key]


def make_in_maps_routed(x, gate_w, gate_proj_w, up_proj_w, down_proj_w):
    """Returns (in_maps, idx_list, n_list, widths)."""
    from concurrent.futures import ThreadPoolExecutor

    x2d = np.ascontiguousarray(np.asarray(x, np.float32).reshape(T, D))
    gating = host_gating(x2d, np.asarray(gate_w, np.float32))  # [T, E]
    idx_list = [np.nonzero(gating[:, c] > 0)[0].astype(np.int64)
                for c in range(N_CORES)]
    n_list = [len(ix) for ix in idx_list]
    widths = chunk_widths(max(n_list))
    cap = sum(widths)

    def prep_core(c):
        ix, n_c = idx_list[c], n_list[c]
        xsel = x2d[ix].T  # [D, n_c]
        xcT = np.zeros((D, cap), dtype=ml_dtypes.bfloat16)
        xcT[:, :n_c] = bf16(xsel)
        xgT = np.zeros((D, cap), dtype=ml_dtypes.bfloat16)
        xgT[:, :n_c] = bf16(xsel * gating[ix, c][None, :])
        return {
            "xcT": xcT,
            "xgT": xgT,
            "wgT": bf16(np.asarray(gate_proj_w[c], np.float32).T),
            "wuT": bf16(np.asarray(up_proj_w[c], np.float32).T),
            "wdT": bf16(np.asarray(down_proj_w[c], np.float32).T),
        }

    with ThreadPoolExecutor(N_CORES) as ex:
        in_maps = list(ex.map(prep_core, range(N_CORES)))
    return in_maps, idx_list, n_list, widths


def kernel(x, gate_w, gate_proj_w, up_proj_w, down_proj_w,
           num_experts_per_tok=2, _trace=False, _trace_cores=None):
    assert int(num_experts_per_tok) == TOPK
    _install_profile_shim()
    from concourse import bass_utils

    kwargs = {}
    if _trace:
        kwargs = dict(trace=True,
                      trace_cores=_trace_cores or list(range(N_CORES)))

    t0 = time.time()
    in_maps, idx_list, n_list, widths = make_in_maps_routed(
        x, gate_w, gate_proj_w, up_proj_w, down_proj_w)
    _log(f"host prep {time.time() - t0:.1f}s (widths={widths}, "
         f"counts={n_list})")
    nc = _get_nc_routed(widths)
    t0 = time.time()
    res = bass_utils.run_bass_kernel_spmd(
        nc, in_maps, core_ids=list(range(N_CORES)), **kwargs)
    _log(f"run_bass_kernel_spmd took {time.time() - t0:.1f}s")
    kernel.last_result = res
    t0 = time.time()
    y = np.zeros((T, D), dtype=np.float32)
    for c in range(N_CORES):
        yc = res.results[c]["ycomp"]  # [D, cap] f16
        y[idx_list[c]] += yc[:, :n_list[c]].astype(np.float32).T
    _log(f"host combine {time.time() - t0:.1f}s")
    return y.reshape(B, S, D)


kernel.last_result = None


# revision 8
# speedup vs baseline: 1.0220x; 1.0220x over previous
"""MoE feed-forward (8 experts, top-2, SwiGLU) on 8 Trainium2 NeuronCores.

Strategy: expert parallelism with host routing. Core c owns expert c and
computes its expert's SwiGLU for the tokens routed to it (host-gathered,
feature-major). All matmul operands are bf16 (rel err ~3e-3, well under the
2e-2 gate) which halves DMA traffic vs fp32r and enables the PE's fast
weight load path. Gating probabilities are folded host-side into a second,
pre-scaled copy of x that feeds the up-projection, so the device applies
gating for free: y_c = Wd @ (silu(Wg x) * (Wu (g*x))). Output is fp16;
host scatter-adds the two expert contributions per token.
"""

import os
import sys
import time

sys.path.insert(0, "/opt/trn_rl_repo")

import numpy as np
import ml_dtypes

# ---------------------------------------------------------------------------
# Problem constants (hardcoded per contract)
B, S, D, E, I, TOPK = 2, 2048, 1024, 8, 1408, 2
T = B * S  # 4096 tokens
P = 128
D_T = D // P   # 8 d-tiles
I_T = I // P   # 11 i-tiles
N_CORES = 8
PSUM_MAX = 512  # fp32 elems per partition per PSUM bank

_VERBOSE = bool(int(os.environ.get("KERNEL_VERBOSE", "0")))


def _log(msg):
    if _VERBOSE:
        print(f"[kernel] {msg}", flush=True)


def _install_profile_shim():
    """Best-effort: make antenv.axon_hooks importable so trace=True works."""
    try:
        import antenv.axon_hooks  # noqa: F401
        return  # real module (or prior shim) present
    except Exception:
        pass
    try:
        sys.path.insert(0, os.path.dirname(os.path.abspath(__file__)))
        import axon_profile_shim
        axon_profile_shim.install()
    except Exception as exc:  # profiling is best-effort
        _log(f"profile shim unavailable: {exc}")


def host_gating(x2d: np.ndarray, gate_w: np.ndarray):
    """Exact router: scores -> top-2 -> softmax. Returns gating [T, E] fp32."""
    scores = x2d.astype(np.float64) @ gate_w.astype(np.float64).T  # [T, E]
    idx = np.argsort(-scores, axis=-1, kind="stable")[:, :TOPK]  # [T, 2]
    top = np.take_along_axis(scores, idx, axis=-1)  # [T, 2] descending
    m = top[:, :1]
    ex = np.exp(top - m)
    probs = ex / ex.sum(axis=-1, keepdims=True)  # [T, 2]
    gating = np.zeros((x2d.shape[0], E), dtype=np.float64)
    np.put_along_axis(gating, idx, probs, axis=-1)
    return gating.astype(np.float32)


def bf16(a: np.ndarray) -> np.ndarray:
    return np.ascontiguousarray(a, dtype=np.float32).astype(ml_dtypes.bfloat16)


def chunk_widths(max_n: int) -> list:
    """Split max_n tokens into PSUM-bank-sized chunks (multiples of 16).
    First chunk maximal (hides the weight-stream window), later chunks
    balanced so the last one (the output-DMA tail) stays small."""
    units = -(-max_n // 16)
    widths = []
    while units * 16 > PSUM_MAX:
        n_left = -(-units * 16 // PSUM_MAX)
        if n_left <= 2:
            break
        widths.append(PSUM_MAX)
        units -= PSUM_MAX // 16
    if units * 16 > PSUM_MAX:
        hi = -(-units // 2)
        widths += [hi * 16, (units - hi) * 16]
    elif units:
        widths.append(units * 16)
    return widths


# ---------------------------------------------------------------------------
# Bass kernel builder


def build_nc_routed(widths, n_cores=N_CORES):
    """Each core computes its expert for `sum(widths)` routed tokens
    (host-gathered, feature-major), in chunks of widths[i] tokens."""
    import concourse.mybir as mybir
    import concourse.tile as tile
    from concourse import bacc

    f32 = mybir.dt.float32
    bf = mybir.dt.bfloat16
    f16 = mybir.dt.float16
    cap = sum(widths)
    w0 = widths[0]

    nc = bacc.Bacc("TRN2", debug=False, num_devices=n_cores)

    xcT_d = nc.dram_tensor("xcT", [D, cap], bf, kind="ExternalInput")
    xgT_d = nc.dram_tensor("xgT", [D, cap], bf, kind="ExternalInput")
    wgT_d = nc.dram_tensor("wgT", [D, I], bf, kind="ExternalInput")
    wuT_d = nc.dram_tensor("wuT", [D, I], bf, kind="ExternalInput")
    wdT_d = nc.dram_tensor("wdT", [I, D], bf, kind="ExternalInput")
    ycomp_d = nc.dram_tensor("ycomp", [D, cap], f16, kind="ExternalOutput")

    xcT_r = xcT_d.ap().rearrange("(do dp) t -> dp do t", dp=P)
    xgT_r = xgT_d.ap().rearrange("(do dp) t -> dp do t", dp=P)
    wgT_r = wgT_d.ap().rearrange("(do dp) i -> dp do i", dp=P)
    wuT_r = wuT_d.ap().rearrange("(do dp) i -> dp do i", dp=P)
    wdT_r = wdT_d.ap().rearrange("(io ip) d -> ip io d", ip=P)
    ycomp_r = ycomp_d.ap().rearrange("(do dp) t -> dp do t", dp=P)

    starts = [sum(widths[:i]) for i in range(len(widths))]

    # i-tile groups sized so live PSUM accumulators stay within 8 banks
    GRPS = [list(range(0, 6)), list(range(6, I_T))]

    with tile.TileContext(nc) as tcx:
        with tcx.tile_pool(name="wpool", bufs=1) as wpool, \
             tcx.tile_pool(name="xpool", bufs=len(widths)) as xpool, \
             tcx.tile_pool(name="xgpool", bufs=len(widths)) as xgpool, \
             tcx.tile_pool(name="hpool", bufs=2) as hpool, \
             tcx.tile_pool(name="ypool", bufs=2) as ypool, \
             tcx.tile_pool(name="gspool", bufs=I_T + 1) as gspool, \
             tcx.tile_pool(name="ps", bufs=8, space="PSUM") as ps:

            wg_sb = wpool.tile([P, D_T, I], bf)
            wu_sb = wpool.tile([P, D_T, I], bf)
            wd_sb = wpool.tile([P, I_T, D], bf)
            xts = [xpool.tile([P, D_T, w0], bf, tag="x", name=f"xt{ci}")
                   for ci in range(len(widths))]
            xgs = [xgpool.tile([P, D_T, w0], bf, tag="x", name=f"xg{ci}")
                   for ci in range(len(widths))]

            # DMA issue order tuned for time-to-first-matmul and the PE's
            # consumption order. The gate/up phases run d-outer, consuming
            # one (x[d], w[d]) row pair per step, so the stream only has to
            # stay one row ahead of the PE. The Sync engine issues the
            # gate-side stream + everything else; the otherwise-idle Act
            # engine issues the up-side stream in parallel.
            for d_o in range(D_T):
                nc.sync.dma_start(xts[0][:, d_o, :widths[0]],
                                  xcT_r[:, d_o, :widths[0]])
                nc.sync.dma_start(wg_sb[:, d_o, :], wgT_r[:, d_o, :])
                nc.scalar.dma_start(xgs[0][:, d_o, :widths[0]],
                                    xgT_r[:, d_o, :widths[0]])
                nc.scalar.dma_start(wu_sb[:, d_o, :], wuT_r[:, d_o, :])
            for i_o in range(I_T):
                nc.sync.dma_start(wd_sb[:, i_o, :], wdT_r[:, i_o, :])
            for ci in range(1, len(widths)):
                t0, w = starts[ci], widths[ci]
                half = D_T // 2
                for s in (0, half):
                    nc.sync.dma_start(xts[ci][:, s:s + half, :w],
                                      xcT_r[:, s:s + half, t0:t0 + w])
                for s in (0, half):
                    nc.sync.dma_start(xgs[ci][:, s:s + half, :w],
                                      xgT_r[:, s:s + half, t0:t0 + w])

            for ci, w in enumerate(widths):
                t0 = starts[ci]
                xt, xg = xts[ci], xgs[ci]

                h = hpool.tile([P, I_T, w0], bf, tag="h")
                gs_tiles = {}
                pgs = {}
                for grp in GRPS:
                    for d_o in range(D_T):
                        for i_o in grp:
                            if d_o == 0:
                                pgs[i_o] = ps.tile([P, w0], f32, tag="ps", name=f"pg{i_o}")
                            nc.tensor.matmul(
                                pgs[i_o][:, :w],
                                wg_sb[:, d_o, i_o * P:(i_o + 1) * P],
                                xt[:, d_o, :w],
                                start=(d_o == 0), stop=(d_o == D_T - 1))
                    for i_o in grp:
                        gs = gspool.tile([P, w0], bf, tag="gs")
                        nc.scalar.activation(gs[:, :w], pgs[i_o][:, :w],
                                             mybir.ActivationFunctionType.Silu)
                        gs_tiles[i_o] = gs
                pus = {}
                for grp in GRPS:
                    for d_o in range(D_T):
                        for i_o in grp:
                            if d_o == 0:
                                pus[i_o] = ps.tile([P, w0], f32, tag="ps", name=f"pu{i_o}")
                            nc.tensor.matmul(
                                pus[i_o][:, :w],
                                wu_sb[:, d_o, i_o * P:(i_o + 1) * P],
                                xg[:, d_o, :w],
                                start=(d_o == 0), stop=(d_o == D_T - 1))
                    for i_o in grp:
                        nc.vector.tensor_mul(out=h[:, i_o, :w],
                                             in0=gs_tiles[i_o][:, :w],
                                             in1=pus[i_o][:, :w])

                yout = ypool.tile([P, D_T, w0], f16, tag="yout")
                for d_o in range(D_T):
                    py = ps.tile([P, w0], f32, tag="ps")
                    for i_o in range(I_T):
                        nc.tensor.matmul(
                            py[:, :w], wd_sb[:, i_o, d_o * P:(d_o + 1) * P],
                            h[:, i_o, :w],
                            start=(i_o == 0), stop=(i_o == I_T - 1))
                    nc.scalar.copy(out=yout[:, d_o, :w], in_=py[:, :w])
                    nc.scalar.dma_start(ycomp_r[:, d_o, t0:t0 + w],
                                        yout[:, d_o, :w])

    nc.compile()
    return nc


# ---------------------------------------------------------------------------
# Host-side wrapper

_CACHED = {}


def _get_nc_routed(widths):
    key = ("routed", tuple(widths))
    if key not in _CACHED:
        t0 = time.time()
        _CACHED[key] = build_nc_routed(list(widths))
        _log(f"built routed bass program (widths={widths}) "
             f"in {time.time() - t0:.1f}s")
    return _CACHED[key]


def make_in_maps_routed(x, gate_w, gate_proj_w, up_proj_w, down_proj_w):
    """Returns (in_maps, idx_list, n_list, widths)."""
    from concurrent.futures import ThreadPoolExecutor

    x2d = np.ascontiguousarray(np.asarray(x, np.float32).reshape(T, D))
    gating = host_gating(x2d, np.asarray(gate_w, np.float32))  # [T, E]
    idx_list = [np.nonzero(gating[:, c] > 0)[0].astype(np.int64)
                for c in range(N_CORES)]
    n_list = [len(ix) for ix in idx_list]
    widths = chunk_widths(max(n_list))
    cap = sum(widths)

    def prep_core(c):
        ix, n_c = idx_list[c], n_list[c]
        xsel = x2d[ix].T  # [D, n_c]
        xcT = np.zeros((D, cap), dtype=ml_dtypes.bfloat16)
        xcT[:, :n_c] = bf16(xsel)
        xgT = np.zeros((D, cap), dtype=ml_dtypes.bfloat16)
        xgT[:, :n_c] = bf16(xsel * gating[ix, c][None, :])
        return {
            "xcT": xcT,
            "xgT": xgT,
            "wgT": bf16(np.asarray(gate_proj_w[c], np.float32).T),
            "wuT": bf16(np.asarray(up_proj_w[c], np.float32).T),
            "wdT": bf16(np.asarray(down_proj_w[c], np.float32).T),
        }

    with ThreadPoolExecutor(N_CORES) as ex:
        in_maps = list(ex.map(prep_core, range(N_CORES)))
    return in_maps, idx_list, n_list, widths


def kernel(x, gate_w, gate_proj_w, up_proj_w, down_proj_w,
           num_experts_per_tok=2, _trace=False, _trace_cores=None):
    assert int(num_experts_per_tok) == TOPK
    _install_profile_shim()
    from concourse import bass_utils

    kwargs = {}
    if _trace:
        kwargs = dict(trace=True,
                      trace_cores=_trace_cores or list(range(N_CORES)))

    t0 = time.time()
    in_maps, idx_list, n_list, widths = make_in_maps_routed(
        x, gate_w, gate_proj_w, up_proj_w, down_proj_w)
    _log(f"host prep {time.time() - t0:.1f}s (widths={widths}, "
         f"counts={n_list})")
    nc = _get_nc_routed(widths)
    t0 = time.time()
    res = bass_utils.run_bass_kernel_spmd(
        nc, in_maps, core_ids=list(range(N_CORES)), **kwargs)
    _log(f"run_bass_kernel_spmd took {time.time() - t0:.1f}s")
    kernel.last_result = res
    t0 = time.time()
    y = np.zeros((T, D), dtype=np.float32)
    for c in range(N_CORES):
        yc = res.results[c]["ycomp"]  # [D, cap] f16
        y[idx_list[c]] += yc[:, :n_list[c]].astype(np.float32).T
    _log(f"host combine {time.time() - t0:.1f}s")
    return y.reshape(B, S, D)


kernel.last_result = None


# revision 9
# speedup vs baseline: 1.0510x; 1.0284x over previous
"""MoE feed-forward (8 experts, top-2, SwiGLU) on 8 Trainium2 NeuronCores.

Strategy: expert parallelism with host routing. Core c owns expert c and
computes its expert's SwiGLU for the tokens routed to it (host-gathered,
feature-major). All matmul operands are bf16 (rel err ~3e-3, well under the
2e-2 gate) which halves DMA traffic vs fp32r and enables the PE's fast
weight load path. Gating probabilities are folded host-side into a second,
pre-scaled copy of x that feeds the up-projection, so the device applies
gating for free: y_c = Wd @ (silu(Wg x) * (Wu (g*x))). Output is fp16;
host scatter-adds the two expert contributions per token.
"""

import os
import sys
import time

sys.path.insert(0, "/opt/trn_rl_repo")

import numpy as np
import ml_dtypes

# ---------------------------------------------------------------------------
# Problem constants (hardcoded per contract)
B, S, D, E, I, TOPK = 2, 2048, 1024, 8, 1408, 2
T = B * S  # 4096 tokens
P = 128
D_T = D // P   # 8 d-tiles
I_T = I // P   # 11 i-tiles
N_CORES = 8
PSUM_MAX = 512  # fp32 elems per partition per PSUM bank

_VERBOSE = bool(int(os.environ.get("KERNEL_VERBOSE", "0")))


def _log(msg):
    if _VERBOSE:
        print(f"[kernel] {msg}", flush=True)


def _install_profile_shim():
    """Best-effort: make antenv.axon_hooks importable so trace=True works."""
    try:
        import antenv.axon_hooks  # noqa: F401
        return  # real module (or prior shim) present
    except Exception:
        pass
    try:
        sys.path.insert(0, os.path.dirname(os.path.abspath(__file__)))
        import axon_profile_shim
        axon_profile_shim.install()
    except Exception as exc:  # profiling is best-effort
        _log(f"profile shim unavailable: {exc}")


def host_gating(x2d: np.ndarray, gate_w: np.ndarray):
    """Exact router: scores -> top-2 -> softmax. Returns gating [T, E] fp32."""
    scores = x2d.astype(np.float64) @ gate_w.astype(np.float64).T  # [T, E]
    idx = np.argsort(-scores, axis=-1, kind="stable")[:, :TOPK]  # [T, 2]
    top = np.take_along_axis(scores, idx, axis=-1)  # [T, 2] descending
    m = top[:, :1]
    ex = np.exp(top - m)
    probs = ex / ex.sum(axis=-1, keepdims=True)  # [T, 2]
    gating = np.zeros((x2d.shape[0], E), dtype=np.float64)
    np.put_along_axis(gating, idx, probs, axis=-1)
    return gating.astype(np.float32)


def bf16(a: np.ndarray) -> np.ndarray:
    return np.ascontiguousarray(a, dtype=np.float32).astype(ml_dtypes.bfloat16)


def chunk_widths(max_n: int) -> list:
    """Split max_n tokens into PSUM-bank-sized chunks (multiples of 16).
    First chunk maximal (hides the weight-stream window), later chunks
    balanced so the last one (the output-DMA tail) stays small."""
    units = -(-max_n // 16)
    widths = []
    while units * 16 > PSUM_MAX:
        n_left = -(-units * 16 // PSUM_MAX)
        if n_left <= 2:
            break
        widths.append(PSUM_MAX)
        units -= PSUM_MAX // 16
    if units * 16 > PSUM_MAX:
        hi = -(-units // 2)
        widths += [hi * 16, (units - hi) * 16]
    elif units:
        widths.append(units * 16)
    return widths


# ---------------------------------------------------------------------------
# Bass kernel builder


def build_nc_routed(widths, n_cores=N_CORES):
    """Each core computes its expert for `sum(widths)` routed tokens
    (host-gathered, feature-major), in chunks of widths[i] tokens."""
    import concourse.mybir as mybir
    import concourse.tile as tile
    from concourse import bacc

    f32 = mybir.dt.float32
    bf = mybir.dt.bfloat16
    f16 = mybir.dt.float16
    cap = sum(widths)
    w0 = widths[0]

    nc = bacc.Bacc("TRN2", debug=False, num_devices=n_cores)

    xcT_d = nc.dram_tensor("xcT", [D, cap], bf, kind="ExternalInput")
    xgT_d = nc.dram_tensor("xgT", [D, cap], bf, kind="ExternalInput")
    wgT_d = nc.dram_tensor("wgT", [D, I], bf, kind="ExternalInput")
    wuT_d = nc.dram_tensor("wuT", [D, I], bf, kind="ExternalInput")
    wdT_d = nc.dram_tensor("wdT", [I, D], bf, kind="ExternalInput")
    ycomp_d = nc.dram_tensor("ycomp", [D, cap], f16, kind="ExternalOutput")

    xcT_r = xcT_d.ap().rearrange("(do dp) t -> dp do t", dp=P)
    xgT_r = xgT_d.ap().rearrange("(do dp) t -> dp do t", dp=P)
    wgT_r = wgT_d.ap().rearrange("(do dp) i -> dp do i", dp=P)
    wuT_r = wuT_d.ap().rearrange("(do dp) i -> dp do i", dp=P)
    wdT_r = wdT_d.ap().rearrange("(io ip) d -> ip io d", ip=P)
    ycomp_r = ycomp_d.ap().rearrange("(do dp) t -> dp do t", dp=P)

    starts = [sum(widths[:i]) for i in range(len(widths))]

    # i-tile groups sized so live PSUM accumulators stay within 8 banks
    GRPS = [list(range(0, 6)), list(range(6, I_T))]

    with tile.TileContext(nc) as tcx:
        with tcx.tile_pool(name="wpool", bufs=1) as wpool, \
             tcx.tile_pool(name="xpool", bufs=len(widths)) as xpool, \
             tcx.tile_pool(name="xgpool", bufs=len(widths)) as xgpool, \
             tcx.tile_pool(name="hpool", bufs=2) as hpool, \
             tcx.tile_pool(name="ypool", bufs=2) as ypool, \
             tcx.tile_pool(name="gspool", bufs=I_T + 1) as gspool, \
             tcx.tile_pool(name="ps", bufs=8, space="PSUM") as ps:

            wg_sb = wpool.tile([P, D_T, I], bf)
            wu_sb = wpool.tile([P, D_T, I], bf)
            wd_sb = wpool.tile([P, I_T, D], bf)
            xts = [xpool.tile([P, D_T, w0], bf, tag="x", name=f"xt{ci}")
                   for ci in range(len(widths))]
            xgs = [xgpool.tile([P, D_T, w0], bf, tag="x", name=f"xg{ci}")
                   for ci in range(len(widths))]

            # DMA issue order tuned for time-to-first-matmul and the PE's
            # consumption order. The gate/up phases run d-outer, consuming
            # one (x[d], w[d]) row pair per step, so the stream only has to
            # stay one row ahead of the PE. Issue cost (~0.7us per dma_start,
            # serialized per engine-sequencer) is the binding constraint, so
            # the streams are spread over the three DMA-capable engines:
            # Sync carries gate->up weights then everything else, Act carries
            # the up-side x (it must be free early for the silus), and the
            # otherwise-idle GpSimd carries the gate-side x.
            for d_o in range(D_T):
                nc.sync.dma_start(wg_sb[:, d_o, :], wgT_r[:, d_o, :])
                nc.gpsimd.dma_start(xts[0][:, d_o, :widths[0]],
                                    xcT_r[:, d_o, :widths[0]])
                nc.scalar.dma_start(xgs[0][:, d_o, :widths[0]],
                                    xgT_r[:, d_o, :widths[0]])
            for d_o in range(D_T):
                nc.sync.dma_start(wu_sb[:, d_o, :], wuT_r[:, d_o, :])
            for i_o in range(I_T):
                nc.sync.dma_start(wd_sb[:, i_o, :], wdT_r[:, i_o, :])
            for ci in range(1, len(widths)):
                t0, w = starts[ci], widths[ci]
                half = D_T // 2
                for s in (0, half):
                    nc.sync.dma_start(xts[ci][:, s:s + half, :w],
                                      xcT_r[:, s:s + half, t0:t0 + w])
                for s in (0, half):
                    nc.sync.dma_start(xgs[ci][:, s:s + half, :w],
                                      xgT_r[:, s:s + half, t0:t0 + w])

            for ci, w in enumerate(widths):
                t0 = starts[ci]
                xt, xg = xts[ci], xgs[ci]

                h = hpool.tile([P, I_T, w0], bf, tag="h")
                gs_tiles = {}
                pgs = {}
                for grp in GRPS:
                    for d_o in range(D_T):
                        for i_o in grp:
                            if d_o == 0:
                                pgs[i_o] = ps.tile([P, w0], f32, tag="ps", name=f"pg{i_o}")
                            nc.tensor.matmul(
                                pgs[i_o][:, :w],
                                wg_sb[:, d_o, i_o * P:(i_o + 1) * P],
                                xt[:, d_o, :w],
                                start=(d_o == 0), stop=(d_o == D_T - 1))
                    for i_o in grp:
                        gs = gspool.tile([P, w0], bf, tag="gs")
                        nc.scalar.activation(gs[:, :w], pgs[i_o][:, :w],
                                             mybir.ActivationFunctionType.Silu)
                        gs_tiles[i_o] = gs
                pus = {}
                for grp in GRPS:
                    for d_o in range(D_T):
                        for i_o in grp:
                            if d_o == 0:
                                pus[i_o] = ps.tile([P, w0], f32, tag="ps", name=f"pu{i_o}")
                            nc.tensor.matmul(
                                pus[i_o][:, :w],
                                wu_sb[:, d_o, i_o * P:(i_o + 1) * P],
                                xg[:, d_o, :w],
                                start=(d_o == 0), stop=(d_o == D_T - 1))
                    for i_o in grp:
                        nc.vector.tensor_mul(out=h[:, i_o, :w],
                                             in0=gs_tiles[i_o][:, :w],
                                             in1=pus[i_o][:, :w])

                yout = ypool.tile([P, D_T, w0], f16, tag="yout")
                for d_o in range(D_T):
                    py = ps.tile([P, w0], f32, tag="ps")
                    for i_o in range(I_T):
                        nc.tensor.matmul(
                            py[:, :w], wd_sb[:, i_o, d_o * P:(d_o + 1) * P],
                            h[:, i_o, :w],
                            start=(i_o == 0), stop=(i_o == I_T - 1))
                    nc.scalar.copy(out=yout[:, d_o, :w], in_=py[:, :w])
                    nc.scalar.dma_start(ycomp_r[:, d_o, t0:t0 + w],
                                        yout[:, d_o, :w])

    nc.compile()
    return nc


# ---------------------------------------------------------------------------
# Host-side wrapper

_CACHED = {}


def _get_nc_routed(widths):
    key = ("routed", tuple(widths))
    if key not in _CACHED:
        t0 = time.time()
        _CACHED[key] = build_nc_routed(list(widths))
        _log(f"built routed bass program (widths={widths}) "
             f"in {time.time() - t0:.1f}s")
    return _CACHED[key]


def make_in_maps_routed(x, gate_w, gate_proj_w, up_proj_w, down_proj_w):
    """Returns (in_maps, idx_list, n_list, widths)."""
    from concurrent.futures import ThreadPoolExecutor

    x2d = np.ascontiguousarray(np.asarray(x, np.float32).reshape(T, D))
    gating = host_gating(x2d, np.asarray(gate_w, np.float32))  # [T, E]
    idx_list = [np.nonzero(gating[:, c] > 0)[0].astype(np.int64)
                for c in range(N_CORES)]
    n_list = [len(ix) for ix in idx_list]
    widths = chunk_widths(max(n_list))
    cap = sum(widths)

    def prep_core(c):
        ix, n_c = idx_list[c], n_list[c]
        xsel = x2d[ix].T  # [D, n_c]
        xcT = np.zeros((D, cap), dtype=ml_dtypes.bfloat16)
        xcT[:, :n_c] = bf16(xsel)
        xgT = np.zeros((D, cap), dtype=ml_dtypes.bfloat16)
        xgT[:, :n_c] = bf16(xsel * gating[ix, c][None, :])
        return {
            "xcT": xcT,
            "xgT": xgT,
            "wgT": bf16(np.asarray(gate_proj_w[c], np.float32).T),
            "wuT": bf16(np.asarray(up_proj_w[c], np.float32).T),
            "wdT": bf16(np.asarray(down_proj_w[c], np.float32).T),
        }

    with ThreadPoolExecutor(N_CORES) as ex:
        in_maps = list(ex.map(prep_core, range(N_CORES)))
    return in_maps, idx_list, n_list, widths


def kernel(x, gate_w, gate_proj_w, up_proj_w, down_proj_w,
           num_experts_per_tok=2, _trace=False, _trace_cores=None):
    assert int(num_experts_per_tok) == TOPK
    _install_profile_shim()
    from concourse import bass_utils

    kwargs = {}
    if _trace:
        kwargs = dict(trace=True,
                      trace_cores=_trace_cores or list(range(N_CORES)))

    t0 = time.time()
    in_maps, idx_list, n_list, widths = make_in_maps_routed(
        x, gate_w, gate_proj_w, up_proj_w, down_proj_w)
    _log(f"host prep {time.time() - t0:.1f}s (widths={widths}, "
         f"counts={n_list})")
    nc = _get_nc_routed(widths)
    t0 = time.time()
    res = bass_utils.run_bass_kernel_spmd(
        nc, in_maps, core_ids=list(range(N_CORES)), **kwargs)
    _log(f"run_bass_kernel_spmd took {time.time() - t0:.1f}s")
    kernel.last_result = res
    t0 = time.time()
    y = np.zeros((T, D), dtype=np.float32)
    for c in range(N_CORES):
        yc = res.results[c]["ycomp"]  # [D, cap] f16
        y[idx_list[c]] += yc[:, :n_list[c]].astype(np.float32).T
    _log(f"host combine {time.time() - t0:.1f}s")
    return y.reshape(B, S, D)


kernel.last_result = None


# revision 11
# speedup vs baseline: 1.0603x; 1.0088x over previous
"""MoE feed-forward (8 experts, top-2, SwiGLU) on 8 Trainium2 NeuronCores.

Strategy: expert parallelism with host routing. Core c owns expert c and
computes its expert's SwiGLU for the tokens routed to it (host-gathered,
feature-major). All matmul operands are bf16 (rel err ~3e-3, well under the
2e-2 gate) which halves DMA traffic vs fp32r and enables the PE's fast
weight load path. Gating probabilities are folded host-side into a second,
pre-scaled copy of x that feeds the up-projection, so the device applies
gating for free: y_c = Wd @ (silu(Wg x) * (Wu (g*x))). Output is fp16;
host scatter-adds the two expert contributions per token.
"""

import os
import sys
import time

sys.path.insert(0, "/opt/trn_rl_repo")

import numpy as np
import ml_dtypes

# ---------------------------------------------------------------------------
# Problem constants (hardcoded per contract)
B, S, D, E, I, TOPK = 2, 2048, 1024, 8, 1408, 2
T = B * S  # 4096 tokens
P = 128
D_T = D // P   # 8 d-tiles
I_T = I // P   # 11 i-tiles
N_CORES = 8
PSUM_MAX = 512  # fp32 elems per partition per PSUM bank

_VERBOSE = bool(int(os.environ.get("KERNEL_VERBOSE", "0")))


def _log(msg):
    if _VERBOSE:
        print(f"[kernel] {msg}", flush=True)


def _install_profile_shim():
    """Best-effort: make antenv.axon_hooks importable so trace=True works."""
    try:
        import antenv.axon_hooks  # noqa: F401
        return  # real module (or prior shim) present
    except Exception:
        pass
    try:
        sys.path.insert(0, os.path.dirname(os.path.abspath(__file__)))
        import axon_profile_shim
        axon_profile_shim.install()
    except Exception as exc:  # profiling is best-effort
        _log(f"profile shim unavailable: {exc}")


def host_gating(x2d: np.ndarray, gate_w: np.ndarray):
    """Exact router: scores -> top-2 -> softmax. Returns gating [T, E] fp32."""
    scores = x2d.astype(np.float64) @ gate_w.astype(np.float64).T  # [T, E]
    idx = np.argsort(-scores, axis=-1, kind="stable")[:, :TOPK]  # [T, 2]
    top = np.take_along_axis(scores, idx, axis=-1)  # [T, 2] descending
    m = top[:, :1]
    ex = np.exp(top - m)
    probs = ex / ex.sum(axis=-1, keepdims=True)  # [T, 2]
    gating = np.zeros((x2d.shape[0], E), dtype=np.float64)
    np.put_along_axis(gating, idx, probs, axis=-1)
    return gating.astype(np.float32)


def bf16(a: np.ndarray) -> np.ndarray:
    return np.ascontiguousarray(a, dtype=np.float32).astype(ml_dtypes.bfloat16)


def chunk_widths(max_n: int) -> list:
    """Split max_n tokens into PSUM-bank-sized chunks (multiples of 16).
    First chunk maximal (hides the weight-stream window), later chunks
    balanced so the last one (the output-DMA tail) stays small."""
    units = -(-max_n // 16)
    widths = []
    while units * 16 > PSUM_MAX:
        n_left = -(-units * 16 // PSUM_MAX)
        if n_left <= 2:
            break
        widths.append(PSUM_MAX)
        units -= PSUM_MAX // 16
    if units * 16 > PSUM_MAX:
        hi = -(-units // 2)
        widths += [hi * 16, (units - hi) * 16]
    elif units:
        widths.append(units * 16)
    return widths


# ---------------------------------------------------------------------------
# Bass kernel builder


def build_nc_routed(widths, n_cores=N_CORES):
    """Each core computes its expert for `sum(widths)` routed tokens
    (host-gathered, feature-major), in chunks of widths[i] tokens."""
    import concourse.mybir as mybir
    import concourse.tile as tile
    from concourse import bacc

    f32 = mybir.dt.float32
    bf = mybir.dt.bfloat16
    f16 = mybir.dt.float16
    cap = sum(widths)
    w0 = widths[0]

    nc = bacc.Bacc("TRN2", debug=False, num_devices=n_cores)

    xcT_d = nc.dram_tensor("xcT", [D, cap], bf, kind="ExternalInput")
    xgT_d = nc.dram_tensor("xgT", [D, cap], bf, kind="ExternalInput")
    wgT_d = nc.dram_tensor("wgT", [D, I], bf, kind="ExternalInput")
    wuT_d = nc.dram_tensor("wuT", [D, I], bf, kind="ExternalInput")
    wdT_d = nc.dram_tensor("wdT", [I, D], bf, kind="ExternalInput")
    ycomp_d = nc.dram_tensor("ycomp", [D, cap], f16, kind="ExternalOutput")

    xcT_r = xcT_d.ap().rearrange("(do dp) t -> dp do t", dp=P)
    xgT_r = xgT_d.ap().rearrange("(do dp) t -> dp do t", dp=P)
    wgT_r = wgT_d.ap().rearrange("(do dp) i -> dp do i", dp=P)
    wuT_r = wuT_d.ap().rearrange("(do dp) i -> dp do i", dp=P)
    wdT_r = wdT_d.ap().rearrange("(io ip) d -> ip io d", ip=P)
    ycomp_r = ycomp_d.ap().rearrange("(do dp) t -> dp do t", dp=P)

    starts = [sum(widths[:i]) for i in range(len(widths))]

    # i-tile groups sized so live PSUM accumulators stay within 8 banks
    GRPS = [list(range(0, 6)), list(range(6, I_T))]

    with tile.TileContext(nc) as tcx:
        with tcx.tile_pool(name="wpool", bufs=1) as wpool, \
             tcx.tile_pool(name="xpool", bufs=len(widths)) as xpool, \
             tcx.tile_pool(name="xgpool", bufs=len(widths)) as xgpool, \
             tcx.tile_pool(name="hpool", bufs=2) as hpool, \
             tcx.tile_pool(name="ypool", bufs=2) as ypool, \
             tcx.tile_pool(name="gspool", bufs=I_T + 1) as gspool, \
             tcx.tile_pool(name="ps", bufs=8, space="PSUM") as ps:

            wg_sb = wpool.tile([P, D_T, I], bf)
            wu_sb = wpool.tile([P, D_T, I], bf)
            wd_sb = wpool.tile([P, I_T, D], bf)
            xts = [xpool.tile([P, D_T, w0], bf, tag="x", name=f"xt{ci}")
                   for ci in range(len(widths))]
            xgs = [xgpool.tile([P, D_T, w0], bf, tag="x", name=f"xg{ci}")
                   for ci in range(len(widths))]

            # DMA issue order tuned for time-to-first-matmul and the PE's
            # consumption order. The gate/up phases run d-outer, consuming
            # one (x[d], w[d]) row pair per step, so the stream only has to
            # stay one row ahead of the PE. Issue cost (~0.7us per dma_start,
            # serialized per engine-sequencer) is the binding constraint, so
            # the streams are spread over the three DMA-capable engines:
            # Sync carries gate->up weights then everything else, Act carries
            # the up-side x (it must be free early for the silus), and the
            # otherwise-idle GpSimd carries the gate-side x.
            # warmup scratch: ramp the PE clock out of its low p-state while
            # the first weight rows stream in (results are never read)
            warm_sb = wpool.tile([P, P], bf)
            nc.vector.memset(warm_sb[:], 0.0)
            warm_ps = [ps.tile([P, w0], f32, tag="ps", name=f"warm{k}")
                       for k in range(4)]
            for k in range(30):
                nc.tensor.matmul(warm_ps[k % 4][:, :P], warm_sb[:],
                                 warm_sb[:], start=True, stop=True)

            ihalf = I // 2
            nc.sync.dma_start(wg_sb[:, 0, :ihalf], wgT_r[:, 0, :ihalf])
            nc.sync.dma_start(wg_sb[:, 0, ihalf:], wgT_r[:, 0, ihalf:])
            for d_o in range(1, D_T):
                nc.sync.dma_start(wg_sb[:, d_o, :], wgT_r[:, d_o, :])
            for d_o in range(D_T):
                nc.scalar.dma_start(xts[0][:, d_o, :widths[0]],
                                    xcT_r[:, d_o, :widths[0]])
                nc.gpsimd.dma_start(xgs[0][:, d_o, :widths[0]],
                                    xgT_r[:, d_o, :widths[0]])
            for d_o in range(D_T):
                nc.sync.dma_start(wu_sb[:, d_o, :], wuT_r[:, d_o, :])
            for i_o in range(I_T):
                nc.sync.dma_start(wd_sb[:, i_o, :], wdT_r[:, i_o, :])
            for ci in range(1, len(widths)):
                t0, w = starts[ci], widths[ci]
                half = D_T // 2
                for s in (0, half):
                    nc.sync.dma_start(xts[ci][:, s:s + half, :w],
                                      xcT_r[:, s:s + half, t0:t0 + w])
                for s in (0, half):
                    nc.sync.dma_start(xgs[ci][:, s:s + half, :w],
                                      xgT_r[:, s:s + half, t0:t0 + w])

            for ci, w in enumerate(widths):
                t0 = starts[ci]
                xt, xg = xts[ci], xgs[ci]

                h = hpool.tile([P, I_T, w0], bf, tag="h")
                gs_tiles = {}
                pgs = {}
                for grp in GRPS:
                    for d_o in range(D_T):
                        for i_o in grp:
                            if d_o == 0:
                                pgs[i_o] = ps.tile([P, w0], f32, tag="ps", name=f"pg{i_o}")
                            nc.tensor.matmul(
                                pgs[i_o][:, :w],
                                wg_sb[:, d_o, i_o * P:(i_o + 1) * P],
                                xt[:, d_o, :w],
                                start=(d_o == 0), stop=(d_o == D_T - 1))
                    for i_o in grp:
                        gs = gspool.tile([P, w0], bf, tag="gs")
                        nc.scalar.activation(gs[:, :w], pgs[i_o][:, :w],
                                             mybir.ActivationFunctionType.Silu)
                        gs_tiles[i_o] = gs
                pus = {}
                for grp in GRPS:
                    for d_o in range(D_T):
                        for i_o in grp:
                            if d_o == 0:
                                pus[i_o] = ps.tile([P, w0], f32, tag="ps", name=f"pu{i_o}")
                            nc.tensor.matmul(
                                pus[i_o][:, :w],
                                wu_sb[:, d_o, i_o * P:(i_o + 1) * P],
                                xg[:, d_o, :w],
                                start=(d_o == 0), stop=(d_o == D_T - 1))
                    for i_o in grp:
                        nc.vector.tensor_mul(out=h[:, i_o, :w],
                                             in0=gs_tiles[i_o][:, :w],
                                             in1=pus[i_o][:, :w])

                yout = ypool.tile([P, D_T, w0], f16, tag="yout")
                for d_o in range(D_T):
                    py = ps.tile([P, w0], f32, tag="ps")
                    for i_o in range(I_T):
                        nc.tensor.matmul(
                            py[:, :w], wd_sb[:, i_o, d_o * P:(d_o + 1) * P],
                            h[:, i_o, :w],
                            start=(i_o == 0), stop=(i_o == I_T - 1))
                    nc.scalar.copy(out=yout[:, d_o, :w], in_=py[:, :w])
                    nc.scalar.dma_start(ycomp_r[:, d_o, t0:t0 + w],
                                        yout[:, d_o, :w])

    nc.compile()
    return nc


# ---------------------------------------------------------------------------
# Host-side wrapper

_CACHED = {}


def _get_nc_routed(widths):
    key = ("routed", tuple(widths))
    if key not in _CACHED:
        t0 = time.time()
        _CACHED[key] = build_nc_routed(list(widths))
        _log(f"built routed bass program (widths={widths}) "
             f"in {time.time() - t0:.1f}s")
    return _CACHED[key]


def make_in_maps_routed(x, gate_w, gate_proj_w, up_proj_w, down_proj_w):
    """Returns (in_maps, idx_list, n_list, widths)."""
    from concurrent.futures import ThreadPoolExecutor

    x2d = np.ascontiguousarray(np.asarray(x, np.float32).reshape(T, D))
    gating = host_gating(x2d, np.asarray(gate_w, np.float32))  # [T, E]
    idx_list = [np.nonzero(gating[:, c] > 0)[0].astype(np.int64)
                for c in range(N_CORES)]
    n_list = [len(ix) for ix in idx_list]
    widths = chunk_widths(max(n_list))
    cap = sum(widths)

    def prep_core(c):
        ix, n_c = idx_list[c], n_list[c]
        xsel = x2d[ix].T  # [D, n_c]
        xcT = np.zeros((D, cap), dtype=ml_dtypes.bfloat16)
        xcT[:, :n_c] = bf16(xsel)
        xgT = np.zeros((D, cap), dtype=ml_dtypes.bfloat16)
        xgT[:, :n_c] = bf16(xsel * gating[ix, c][None, :])
        return {
            "xcT": xcT,
            "xgT": xgT,
            "wgT": bf16(np.asarray(gate_proj_w[c], np.float32).T),
            "wuT": bf16(np.asarray(up_proj_w[c], np.float32).T),
            "wdT": bf16(np.asarray(down_proj_w[c], np.float32).T),
        }

    with ThreadPoolExecutor(N_CORES) as ex:
        in_maps = list(ex.map(prep_core, range(N_CORES)))
    return in_maps, idx_list, n_list, widths


def kernel(x, gate_w, gate_proj_w, up_proj_w, down_proj_w,
           num_experts_per_tok=2, _trace=False, _trace_cores=None):
    assert int(num_experts_per_tok) == TOPK
    _install_profile_shim()
    from concourse import bass_utils

    kwargs = {}
    if _trace:
        kwargs = dict(trace=True,
                      trace_cores=_trace_cores or list(range(N_CORES)))

    t0 = time.time()
    in_maps, idx_list, n_list, widths = make_in_maps_routed(
        x, gate_w, gate_proj_w, up_proj_w, down_proj_w)
    _log(f"host prep {time.time() - t0:.1f}s (widths={widths}, "
         f"counts={n_list})")
    nc = _get_nc_routed(widths)
    t0 = time.time()
    res = bass_utils.run_bass_kernel_spmd(
        nc, in_maps, core_ids=list(range(N_CORES)), **kwargs)
    _log(f"run_bass_kernel_spmd took {time.time() - t0:.1f}s")
    kernel.last_result = res
    t0 = time.time()
    y = np.zeros((T, D), dtype=np.float32)
    for c in range(N_CORES):
        yc = res.results[c]["ycomp"]  # [D, cap] f16
        y[idx_list[c]] += yc[:, :n_list[c]].astype(np.float32).T
    _log(f"host combine {time.time() - t0:.1f}s")
    return y.reshape(B, S, D)


kernel.last_result = None
